# revision 6
# baseline (speedup 1.0000x reference)
"""Multi-head attention forward kernel for Trainium2 (8 NeuronCores).

Problem: B=2, N=2048, C=1024, H=16 heads, head_dim=64.
    q = x @ Wq.T + bq  (same for k, v)
    out = softmax(q k^T / sqrt(C)) v       (per head), re-merged to [B, N, C]

Sharding: core = (batch b, head-group g): b = core // 4, g = core % 4.
Each core computes 4 heads of one batch element. No collectives needed --
outputs are disjoint; host gathers and finishes with a cheap epilogue
(normalize by the fused row-sums and transpose).

Per-core kernel layout (all "T" tensors have the contraction/partition dim
first):
    xt  [C=1024, N=2048]   x[b] transposed (host-side)
    w*t [C=1024, 256]      W[heads-slice].T (host-side)
    QT, KT [256, N]        projections in head-major layout: rows = 4*64
    V   [N, 4, 65]         natural layout + fused ones column (row-sums)
    S^T chunk [128 keys, 512 queries] per (head, qb, kb), exp'd on ACT,
    O^T [65, 512] = [V|1]^T @ P^T accumulated over 16 key chunks in PSUM.
Output "out" [4, 65, N]: per head 64 rows of unnormalized O^T + 1 row of
softmax denominators.
"""

import os
import sys

import ml_dtypes
import numpy as np

for _p in ("/opt/trn_rl_repo",):
    if _p not in sys.path:
        sys.path.insert(0, _p)

import concourse.bass as bass  # noqa: E402
import concourse.tile as tile  # noqa: E402
from concourse import bacc, mybir  # noqa: E402
from concourse.bass_utils import run_bass_kernel_spmd  # noqa: E402

N = 2048  # sequence length
C = 1024  # model dim
D = 64  # head dim
NH = 4  # heads per core
HD = NH * D  # 256 output channels per core
NCORES = 8
KB = N // 128  # 16 key chunks of 128
QB = N // 512  # 4 query blocks of 512
KC = C // 128  # 8 contraction chunks for projections
SCALE = 1.0 / 32.0  # 1 / sqrt(C)

F32 = mybir.dt.float32
F32R = mybir.dt.float32r
BF16 = mybir.dt.bfloat16


def build_kernel(tc, xt, wqt, wkt, wvt, bq, bk, bv, out):
    nc = tc.nc
    Exp = mybir.ActivationFunctionType.Exp

    with tc.tile_pool(name="res", bufs=1) as res:
        # ---- resident SBUF tensors ----
        xt_sb = [res.tile([128, N], BF16, tag=f"xt{k}", name=f"xt{k}") for k in range(KC)]
        wq_sb = [res.tile([128, HD], BF16, tag=f"wq{k}", name=f"wq{k}") for k in range(KC)]
        wk_sb = [res.tile([128, HD], BF16, tag=f"wk{k}", name=f"wk{k}") for k in range(KC)]
        wv_sb = [res.tile([128, HD], BF16, tag=f"wv{k}", name=f"wv{k}") for k in range(KC)]
        qt_sb = [res.tile([128, N], BF16, tag=f"qt{m}", name=f"qt{m}") for m in range(2)]
        kt_sb = [res.tile([128, N], BF16, tag=f"kt{m}", name=f"kt{m}") for m in range(2)]
        v_sb = [res.tile([128, NH, D + 1], BF16, tag=f"v{kb}", name=f"v{kb}") for kb in range(KB)]
        bq_sb = [res.tile([128, 1], F32, tag=f"bq{m}", name=f"bq{m}") for m in range(2)]
        bk_sb = [res.tile([128, 1], F32, tag=f"bk{m}", name=f"bk{m}") for m in range(2)]
        bv_sb = res.tile([128, HD], F32, tag="bv", name="bv")

        # ---- input DMAs ----
        for k in range(KC):
            sl = slice(k * 128, (k + 1) * 128)
            nc.sync.dma_start(out=xt_sb[k][:], in_=xt[sl, :])
            nc.sync.dma_start(out=wq_sb[k][:], in_=wqt[sl, :])
            nc.sync.dma_start(out=wk_sb[k][:], in_=wkt[sl, :])
            nc.sync.dma_start(out=wv_sb[k][:], in_=wvt[sl, :])
        for m in range(2):
            sl = slice(m * 128, (m + 1) * 128)
            nc.sync.dma_start(out=bq_sb[m][:], in_=bq[sl])
            nc.sync.dma_start(out=bk_sb[m][:], in_=bk[sl])
        bv_bcast = bass.AP(tensor=bv.tensor, offset=bv.offset, ap=[[0, 128]] + list(bv.ap))
        nc.sync.dma_start(out=bv_sb[:], in_=bv_bcast)

        # ---- projections ----
        with tc.tile_pool(name="ppsum", bufs=3, space="PSUM") as ppsum:
            # V = x @ Wv.T + bv, natural [keys, channels] layout, with a
            # fused ones column per head for the softmax denominators.
            for kb in range(KB):
                vps = ppsum.tile([128, HD], F32, tag="vps", name="vps")
                for k in range(KC):
                    nc.tensor.matmul(
                        out=vps[:],
                        lhsT=xt_sb[k][:, kb * 128 : (kb + 1) * 128],
                        rhs=wv_sb[k][:],
                        start=(k == 0),
                        stop=(k == KC - 1),
                    )
                nc.vector.tensor_add(
                    out=v_sb[kb][:, :, 0:D],
                    in0=vps[:].rearrange("p (h d) -> p h d", h=NH),
                    in1=bv_sb[:].rearrange("p (h d) -> p h d", h=NH),
                )
                nc.vector.memset(v_sb[kb][:, :, D : D + 1], 1.0)

            # QT/KT = (x @ W.T + b).T, head-major [256, N] layout.
            for w_sb, b_sb, t_sb in ((wq_sb, bq_sb, qt_sb), (wk_sb, bk_sb, kt_sb)):
                for m in range(2):
                    for nb in range(QB):
                        nsl = slice(nb * 512, (nb + 1) * 512)
                        ps = ppsum.tile([128, 512], F32, tag="qkps", name="qkps")
                        for k in range(KC):
                            nc.tensor.matmul(
                                out=ps[:],
                                lhsT=w_sb[k][:, m * 128 : (m + 1) * 128],
                                rhs=xt_sb[k][:, nsl],
                                start=(k == 0),
                                stop=(k == KC - 1),
                            )
                        nc.vector.tensor_scalar_add(
                            out=t_sb[m][:, nsl], in0=ps[:], scalar1=b_sb[m][:]
                        )

        # ---- attention ----
        with (
            tc.tile_pool(name="stp", bufs=2, space="PSUM") as stp,
            tc.tile_pool(name="opp", bufs=2, space="PSUM") as opp,
            tc.tile_pool(name="ptp", bufs=3) as ptp,
            tc.tile_pool(name="otp", bufs=4) as otp,
        ):
            for p in range(2):  # head pairs (2p, 2p+1)
                for qb in range(QB):
                    qsl = slice(qb * 512, (qb + 1) * 512)
                    o_ps = [
                        opp.tile([D + 1, 512], F32, tag=f"o{h}", name=f"o{h}") for h in range(2)
                    ]
                    for kb in range(KB):
                        ksl = slice(kb * 128, (kb + 1) * 128)
                        st = stp.tile([128, 1024], F32, tag="st", name="st")
                        # S^T chunks for both heads; K=64 matmuls at row
                        # offsets 0 and 64 run concurrently on the PE.
                        for h in range(2):
                            hsl = slice(h * D, (h + 1) * D)
                            nc.tensor.matmul(
                                out=st[:, h * 512 : (h + 1) * 512],
                                lhsT=kt_sb[p][hsl, ksl],
                                rhs=qt_sb[p][hsl, qsl],
                                start=True,
                                stop=True,
                            )
                        pt = ptp.tile([128, 1024], BF16, tag="pt", name="pt")
                        nc.scalar.activation(out=pt[:], in_=st[:], func=Exp, scale=SCALE)
                        for h in range(2):
                            nc.tensor.matmul(
                                out=o_ps[h][:],
                                lhsT=v_sb[kb][:, 2 * p + h, :],
                                rhs=pt[:, h * 512 : (h + 1) * 512],
                                start=(kb == 0),
                                stop=(kb == KB - 1),
                                skip_group_check=True,
                            )
                    for h in range(2):
                        ot = otp.tile([D + 1, 512], F32, tag="ot", name="ot")
                        nc.vector.tensor_copy(out=ot[:], in_=o_ps[h][:])
                        nc.sync.dma_start(out=out[2 * p + h, :, qsl], in_=ot[:])


def build_nc():
    nc = bacc.Bacc(
        "TRN2",
        target_bir_lowering=False,
        debug=False,
        num_devices=NCORES,
        enable_partition_id=False,
    )
    xt = nc.dram_tensor("xt", [C, N], BF16, kind="ExternalInput").ap()
    wqt = nc.dram_tensor("wqt", [C, HD], BF16, kind="ExternalInput").ap()
    wkt = nc.dram_tensor("wkt", [C, HD], BF16, kind="ExternalInput").ap()
    wvt = nc.dram_tensor("wvt", [C, HD], BF16, kind="ExternalInput").ap()
    bq = nc.dram_tensor("bq", [HD], F32, kind="ExternalInput").ap()
    bk = nc.dram_tensor("bk", [HD], F32, kind="ExternalInput").ap()
    bv = nc.dram_tensor("bv", [HD], F32, kind="ExternalInput").ap()
    out = nc.dram_tensor("out", [NH, D + 1, N], F32, kind="ExternalOutput").ap()

    with tile.TileContext(nc) as tc:
        build_kernel(tc, xt, wqt, wkt, wvt, bq, bk, bv, out)
    nc.compile()
    return nc


def shard_inputs(inputs):
    x = np.asarray(inputs["x"], np.float32)
    in_maps = []
    for core in range(NCORES):
        b, g = core // 4, core % 4
        sl = slice(g * HD, (g + 1) * HD)
        in_maps.append(
            {
                "xt": np.ascontiguousarray(x[b].T).astype(ml_dtypes.bfloat16),
                "wqt": np.ascontiguousarray(np.asarray(inputs["Wq"], np.float32)[sl, :].T).astype(ml_dtypes.bfloat16),
                "wkt": np.ascontiguousarray(np.asarray(inputs["Wk"], np.float32)[sl, :].T).astype(ml_dtypes.bfloat16),
                "wvt": np.ascontiguousarray(np.asarray(inputs["Wv"], np.float32)[sl, :].T).astype(ml_dtypes.bfloat16),
                "bq": np.ascontiguousarray(np.asarray(inputs["bq"], np.float32)[sl]),
                "bk": np.ascontiguousarray(np.asarray(inputs["bk"], np.float32)[sl]),
                "bv": np.ascontiguousarray(np.asarray(inputs["bv"], np.float32)[sl]),
            }
        )
    return in_maps


def assemble(results, B=2):
    out = np.zeros((B, N, C), np.float32)
    for core in range(NCORES):
        b, g = core // 4, core % 4
        o = np.asarray(results[core]["out"], np.float32)  # [NH, D+1, N]
        on = o[:, 0:D, :] / o[:, D : D + 1, :]  # normalize by row-sums
        # [h, d, n] -> [n, h*D + d]
        out[b, :, g * HD : (g + 1) * HD] = (
            on.transpose(2, 0, 1).reshape(N, HD)
        )
    return out


_NC_CACHE = None


def _get_nc():
    global _NC_CACHE
    if _NC_CACHE is None:
        _NC_CACHE = build_nc()
    return _NC_CACHE


def kernel(**inputs):
    nc = _get_nc()
    in_maps = shard_inputs(inputs)
    res = run_bass_kernel_spmd(
        nc,
        in_maps,
        core_ids=list(range(NCORES)),
        trace=bool(int(os.environ.get("KERNEL_TRACE", "0"))),
    )
    return assemble(res.results, B=int(np.asarray(inputs["x"]).shape[0]))


# revision 7
# speedup vs baseline: 1.0302x; 1.0302x over previous
"""Multi-head attention forward kernel for Trainium2 (8 NeuronCores).

Problem: B=2, N=2048, C=1024, H=16 heads, head_dim=64.
    q = x @ Wq.T + bq  (same for k, v)
    out = softmax(q k^T / sqrt(C)) v       (per head), re-merged to [B, N, C]

Sharding: core = (batch b, head-group g): b = core // 4, g = core % 4.
Each core computes 4 heads of one batch element. No collectives needed --
outputs are disjoint; host gathers and finishes with a cheap epilogue
(normalize by the fused row-sums and transpose).

Per-core kernel layout (all "T" tensors have the contraction/partition dim
first):
    xt  [C=1024, N=2048]   x[b] transposed (host-side)
    w*t [C=1024, 256]      W[heads-slice].T (host-side)
    QT, KT [256, N]        projections in head-major layout: rows = 4*64
    V   [N, 4, 65]         natural layout + fused ones column (row-sums)
    S^T chunk [128 keys, 512 queries] per (head, qb, kb), exp'd on ACT,
    O^T [65, 512] = [V|1]^T @ P^T accumulated over 16 key chunks in PSUM.
Output "out" [4, 65, N]: per head 64 rows of unnormalized O^T + 1 row of
softmax denominators.
"""

import os
import sys

import ml_dtypes
import numpy as np

for _p in ("/opt/trn_rl_repo",):
    if _p not in sys.path:
        sys.path.insert(0, _p)

import concourse.bass as bass  # noqa: E402
import concourse.tile as tile  # noqa: E402
from concourse import bacc, mybir  # noqa: E402
from concourse.bass_utils import run_bass_kernel_spmd  # noqa: E402

N = 2048  # sequence length
C = 1024  # model dim
D = 64  # head dim
NH = 4  # heads per core
HD = NH * D  # 256 output channels per core
NCORES = 8
KB = N // 128  # 16 key chunks of 128
QB = N // 512  # 4 query blocks of 512
KC = C // 128  # 8 contraction chunks for projections
SCALE = 1.0 / 32.0  # 1 / sqrt(C)

F32 = mybir.dt.float32
F32R = mybir.dt.float32r
BF16 = mybir.dt.bfloat16


def build_kernel(tc, xt, wqt, wkt, wvt, bq, bk, bv, out):
    nc = tc.nc
    Exp = mybir.ActivationFunctionType.Exp

    with tc.tile_pool(name="res", bufs=1) as res:
        # ---- resident SBUF tensors ----
        xt_sb = [res.tile([128, N], BF16, tag=f"xt{k}", name=f"xt{k}") for k in range(KC)]
        wq_sb = [res.tile([128, HD], BF16, tag=f"wq{k}", name=f"wq{k}") for k in range(KC)]
        wk_sb = [res.tile([128, HD], BF16, tag=f"wk{k}", name=f"wk{k}") for k in range(KC)]
        wv_sb = [res.tile([128, HD], BF16, tag=f"wv{k}", name=f"wv{k}") for k in range(KC)]
        qt_sb = [res.tile([128, N], BF16, tag=f"qt{m}", name=f"qt{m}") for m in range(2)]
        kt_sb = [res.tile([128, N], BF16, tag=f"kt{m}", name=f"kt{m}") for m in range(2)]
        v_sb = [res.tile([128, NH, D + 1], BF16, tag=f"v{kb}", name=f"v{kb}") for kb in range(KB)]
        bq_sb = [res.tile([128, 1], F32, tag=f"bq{m}", name=f"bq{m}") for m in range(2)]
        bk_sb = [res.tile([128, 1], F32, tag=f"bk{m}", name=f"bk{m}") for m in range(2)]
        bv_sb = res.tile([128, HD], F32, tag="bv", name="bv")

        # ---- input DMAs ----
        for k in range(KC):
            sl = slice(k * 128, (k + 1) * 128)
            nc.sync.dma_start(out=xt_sb[k][:], in_=xt[sl, :])
            nc.sync.dma_start(out=wq_sb[k][:], in_=wqt[sl, :])
            nc.sync.dma_start(out=wk_sb[k][:], in_=wkt[sl, :])
            nc.sync.dma_start(out=wv_sb[k][:], in_=wvt[sl, :])
        for m in range(2):
            sl = slice(m * 128, (m + 1) * 128)
            nc.sync.dma_start(out=bq_sb[m][:], in_=bq[sl])
            nc.sync.dma_start(out=bk_sb[m][:], in_=bk[sl])
        bv_bcast = bass.AP(tensor=bv.tensor, offset=bv.offset, ap=[[0, 128]] + list(bv.ap))
        nc.sync.dma_start(out=bv_sb[:], in_=bv_bcast)

        # ---- projections + attention, interleaved per head pair ----
        # Emission (= scheduler priority) order: Q/K pair 0, V, attention
        # pair 0, Q/K pair 1 (PE filler under pair-0's ACT-bound window),
        # attention pair 1.  PSUM budget: st 2x2 + o 2x1 + proj 2 = 8 banks.
        with (
            tc.tile_pool(name="ppsum", bufs=2, space="PSUM") as ppsum,
            tc.tile_pool(name="stp", bufs=2, space="PSUM") as stp,
            tc.tile_pool(name="opp", bufs=1, space="PSUM") as opp,
            tc.tile_pool(name="ptp", bufs=3) as ptp,
            tc.tile_pool(name="otp", bufs=4) as otp,
        ):

            def proj_qk(m):
                for w_sb, b_sb, t_sb in ((wq_sb, bq_sb, qt_sb), (wk_sb, bk_sb, kt_sb)):
                    for nb in range(QB):
                        nsl = slice(nb * 512, (nb + 1) * 512)
                        ps = ppsum.tile([128, 512], F32, tag="qkps", name="qkps")
                        for k in range(KC):
                            nc.tensor.matmul(
                                out=ps[:],
                                lhsT=w_sb[k][:, m * 128 : (m + 1) * 128],
                                rhs=xt_sb[k][:, nsl],
                                start=(k == 0),
                                stop=(k == KC - 1),
                            )
                        nc.vector.tensor_scalar_add(
                            out=t_sb[m][:, nsl], in0=ps[:], scalar1=b_sb[m][:]
                        )

            def proj_v():
                # V = x @ Wv.T + bv, natural [keys, channels] layout, with a
                # fused ones column per head for the softmax denominators.
                for kb in range(KB):
                    vps = ppsum.tile([128, HD], F32, tag="qkps", name="vps")
                    for k in range(KC):
                        nc.tensor.matmul(
                            out=vps[:],
                            lhsT=xt_sb[k][:, kb * 128 : (kb + 1) * 128],
                            rhs=wv_sb[k][:],
                            start=(k == 0),
                            stop=(k == KC - 1),
                        )
                    nc.vector.tensor_add(
                        out=v_sb[kb][:, :, 0:D],
                        in0=vps[:].rearrange("p (h d) -> p h d", h=NH),
                        in1=bv_sb[:].rearrange("p (h d) -> p h d", h=NH),
                    )
                    nc.vector.memset(v_sb[kb][:, :, D : D + 1], 1.0)

            def attn(p):
                for qb in range(QB):
                    qsl = slice(qb * 512, (qb + 1) * 512)
                    o_ps = [
                        opp.tile([D + 1, 512], F32, tag=f"o{h}", name=f"o{h}")
                        for h in range(2)
                    ]
                    for kb in range(KB):
                        ksl = slice(kb * 128, (kb + 1) * 128)
                        st = stp.tile([128, 1024], F32, tag="st", name="st")
                        # S^T chunks for both heads; K=64 matmuls at row
                        # offsets 0 and 64 run concurrently on the PE.
                        for h in range(2):
                            hsl = slice(h * D, (h + 1) * D)
                            nc.tensor.matmul(
                                out=st[:, h * 512 : (h + 1) * 512],
                                lhsT=kt_sb[p][hsl, ksl],
                                rhs=qt_sb[p][hsl, qsl],
                                start=True,
                                stop=True,
                            )
                        pt = ptp.tile([128, 1024], BF16, tag="pt", name="pt")
                        nc.scalar.activation(
                            out=pt[:], in_=st[:], func=Exp, scale=SCALE
                        )
                        for h in range(2):
                            nc.tensor.matmul(
                                out=o_ps[h][:],
                                lhsT=v_sb[kb][:, 2 * p + h, :],
                                rhs=pt[:, h * 512 : (h + 1) * 512],
                                start=(kb == 0),
                                stop=(kb == KB - 1),
                                skip_group_check=True,
                            )
                    for h in range(2):
                        ot = otp.tile([D + 1, 512], F32, tag="ot", name="ot")
                        nc.vector.tensor_copy(out=ot[:], in_=o_ps[h][:])
                        nc.sync.dma_start(out=out[2 * p + h, :, qsl], in_=ot[:])

            proj_qk(0)
            proj_v()
            attn(0)
            proj_qk(1)
            attn(1)


def build_nc():
    nc = bacc.Bacc(
        "TRN2",
        target_bir_lowering=False,
        debug=False,
        num_devices=NCORES,
        enable_partition_id=False,
    )
    xt = nc.dram_tensor("xt", [C, N], BF16, kind="ExternalInput").ap()
    wqt = nc.dram_tensor("wqt", [C, HD], BF16, kind="ExternalInput").ap()
    wkt = nc.dram_tensor("wkt", [C, HD], BF16, kind="ExternalInput").ap()
    wvt = nc.dram_tensor("wvt", [C, HD], BF16, kind="ExternalInput").ap()
    bq = nc.dram_tensor("bq", [HD], F32, kind="ExternalInput").ap()
    bk = nc.dram_tensor("bk", [HD], F32, kind="ExternalInput").ap()
    bv = nc.dram_tensor("bv", [HD], F32, kind="ExternalInput").ap()
    out = nc.dram_tensor("out", [NH, D + 1, N], F32, kind="ExternalOutput").ap()

    with tile.TileContext(nc) as tc:
        build_kernel(tc, xt, wqt, wkt, wvt, bq, bk, bv, out)
    nc.compile()
    return nc


def shard_inputs(inputs):
    x = np.asarray(inputs["x"], np.float32)
    in_maps = []
    for core in range(NCORES):
        b, g = core // 4, core % 4
        sl = slice(g * HD, (g + 1) * HD)
        in_maps.append(
            {
                "xt": np.ascontiguousarray(x[b].T).astype(ml_dtypes.bfloat16),
                "wqt": np.ascontiguousarray(np.asarray(inputs["Wq"], np.float32)[sl, :].T).astype(ml_dtypes.bfloat16),
                "wkt": np.ascontiguousarray(np.asarray(inputs["Wk"], np.float32)[sl, :].T).astype(ml_dtypes.bfloat16),
                "wvt": np.ascontiguousarray(np.asarray(inputs["Wv"], np.float32)[sl, :].T).astype(ml_dtypes.bfloat16),
                "bq": np.ascontiguousarray(np.asarray(inputs["bq"], np.float32)[sl]),
                "bk": np.ascontiguousarray(np.asarray(inputs["bk"], np.float32)[sl]),
                "bv": np.ascontiguousarray(np.asarray(inputs["bv"], np.float32)[sl]),
            }
        )
    return in_maps


def assemble(results, B=2):
    out = np.zeros((B, N, C), np.float32)
    for core in range(NCORES):
        b, g = core // 4, core % 4
        o = np.asarray(results[core]["out"], np.float32)  # [NH, D+1, N]
        on = o[:, 0:D, :] / o[:, D : D + 1, :]  # normalize by row-sums
        # [h, d, n] -> [n, h*D + d]
        out[b, :, g * HD : (g + 1) * HD] = (
            on.transpose(2, 0, 1).reshape(N, HD)
        )
    return out


_NC_CACHE = None


def _get_nc():
    global _NC_CACHE
    if _NC_CACHE is None:
        _NC_CACHE = build_nc()
    return _NC_CACHE


def kernel(**inputs):
    nc = _get_nc()
    in_maps = shard_inputs(inputs)
    res = run_bass_kernel_spmd(
        nc,
        in_maps,
        core_ids=list(range(NCORES)),
        trace=bool(int(os.environ.get("KERNEL_TRACE", "0"))),
    )
    return assemble(res.results, B=int(np.asarray(inputs["x"]).shape[0]))


# revision 8
# speedup vs baseline: 1.0673x; 1.0360x over previous
"""Multi-head attention forward kernel for Trainium2 (8 NeuronCores).

Problem: B=2, N=2048, C=1024, H=16 heads, head_dim=64.
    q = x @ Wq.T + bq  (same for k, v)
    out = softmax(q k^T / sqrt(C)) v       (per head), re-merged to [B, N, C]

Sharding: core = (batch b, head-group g): b = core // 4, g = core % 4.
Each core computes 4 heads of one batch element. No collectives needed --
outputs are disjoint; host gathers and finishes with a cheap epilogue
(normalize by the fused row-sums and transpose).

Per-core kernel layout (all "T" tensors have the contraction/partition dim
first):
    xt  [C=1024, N=2048]   x[b] transposed (host-side)
    w*t [C=1024, 256]      W[heads-slice].T (host-side)
    QT, KT [256, N]        projections in head-major layout: rows = 4*64
    V   [N, 4, 65]         natural layout + fused ones column (row-sums)
    S^T chunk [128 keys, 512 queries] per (head, qb, kb), exp'd on ACT,
    O^T [65, 512] = [V|1]^T @ P^T accumulated over 16 key chunks in PSUM.
Output "out" [4, 65, N]: per head 64 rows of unnormalized O^T + 1 row of
softmax denominators.
"""

import os
import sys

import ml_dtypes
import numpy as np

for _p in ("/opt/trn_rl_repo",):
    if _p not in sys.path:
        sys.path.insert(0, _p)

import concourse.bass as bass  # noqa: E402
import concourse.tile as tile  # noqa: E402
from concourse import bacc, mybir  # noqa: E402
from concourse.bass_utils import run_bass_kernel_spmd  # noqa: E402

N = 2048  # sequence length
C = 1024  # model dim
D = 64  # head dim
NH = 4  # heads per core
HD = NH * D  # 256 output channels per core
NCORES = 8
KB = N // 128  # 16 key chunks of 128
QB = N // 512  # 4 query blocks of 512
KC = C // 128  # 8 contraction chunks for projections
SCALE = 1.0 / 32.0  # 1 / sqrt(C)

F32 = mybir.dt.float32
F32R = mybir.dt.float32r
BF16 = mybir.dt.bfloat16


def build_kernel(tc, xt, wqt, wkt, wvt, bq, bk, bv, out):
    nc = tc.nc
    Exp = mybir.ActivationFunctionType.Exp

    with tc.tile_pool(name="res", bufs=1) as res:
        # ---- resident SBUF tensors ----
        xt_all = res.tile([128, KC, N], BF16, tag="xt", name="xt")
        wq_all = res.tile([128, KC, HD], BF16, tag="wq", name="wq")
        wk_all = res.tile([128, KC, HD], BF16, tag="wk", name="wk")
        wv_all = res.tile([128, KC, HD], BF16, tag="wv", name="wv")
        xt_sb = [xt_all[:, k, :] for k in range(KC)]
        wq_sb = [wq_all[:, k, :] for k in range(KC)]
        wk_sb = [wk_all[:, k, :] for k in range(KC)]
        wv_sb = [wv_all[:, k, :] for k in range(KC)]
        qt_sb = [res.tile([128, N], BF16, tag=f"qt{m}", name=f"qt{m}") for m in range(2)]
        kt_sb = [res.tile([128, N], BF16, tag=f"kt{m}", name=f"kt{m}") for m in range(2)]
        v_sb = [res.tile([128, NH, D + 1], BF16, tag=f"v{kb}", name=f"v{kb}") for kb in range(KB)]
        bq_sb = [res.tile([128, 1], F32, tag=f"bq{m}", name=f"bq{m}") for m in range(2)]
        bk_sb = [res.tile([128, 1], F32, tag=f"bk{m}", name=f"bk{m}") for m in range(2)]
        bv_sb = res.tile([128, HD], F32, tag="bv", name="bv")

        # ---- input DMAs (one per tensor; xt first -- projections need it) ----
        nc.sync.dma_start(out=xt_all[:], in_=xt.rearrange("(k p) n -> p k n", p=128))
        nc.sync.dma_start(out=wq_all[:], in_=wqt.rearrange("(k p) n -> p k n", p=128))
        nc.sync.dma_start(out=wk_all[:], in_=wkt.rearrange("(k p) n -> p k n", p=128))
        nc.sync.dma_start(out=wv_all[:], in_=wvt.rearrange("(k p) n -> p k n", p=128))
        for m in range(2):
            sl = slice(m * 128, (m + 1) * 128)
            nc.sync.dma_start(out=bq_sb[m][:], in_=bq[sl])
            nc.sync.dma_start(out=bk_sb[m][:], in_=bk[sl])
        bv_bcast = bass.AP(tensor=bv.tensor, offset=bv.offset, ap=[[0, 128]] + list(bv.ap))
        nc.sync.dma_start(out=bv_sb[:], in_=bv_bcast)

        # ---- projections + attention, interleaved per head pair ----
        # Emission (= scheduler priority) order: Q/K pair 0, V, attention
        # pair 0, Q/K pair 1 (PE filler under pair-0's ACT-bound window),
        # attention pair 1.  PSUM budget: st 2x2 + o 2x1 + proj 2 = 8 banks.
        with (
            tc.tile_pool(name="ppsum", bufs=2, space="PSUM") as ppsum,
            tc.tile_pool(name="stp", bufs=2, space="PSUM") as stp,
            tc.tile_pool(name="opp", bufs=1, space="PSUM") as opp,
            tc.tile_pool(name="ptp", bufs=3) as ptp,
            tc.tile_pool(name="otp", bufs=4) as otp,
        ):

            def proj_qk(m):
                for w_sb, b_sb, t_sb in ((wq_sb, bq_sb, qt_sb), (wk_sb, bk_sb, kt_sb)):
                    for nb in range(QB):
                        nsl = slice(nb * 512, (nb + 1) * 512)
                        ps = ppsum.tile([128, 512], F32, tag="qkps", name="qkps")
                        for k in range(KC):
                            nc.tensor.matmul(
                                out=ps[:],
                                lhsT=w_sb[k][:, m * 128 : (m + 1) * 128],
                                rhs=xt_sb[k][:, nsl],
                                start=(k == 0),
                                stop=(k == KC - 1),
                            )
                        nc.vector.tensor_scalar_add(
                            out=t_sb[m][:, nsl], in0=ps[:], scalar1=b_sb[m][:]
                        )

            def proj_v():
                # V = x @ Wv.T + bv, natural [keys, channels] layout, with a
                # fused ones column per head for the softmax denominators.
                for kb in range(KB):
                    vps = ppsum.tile([128, HD], F32, tag="qkps", name="vps")
                    for k in range(KC):
                        nc.tensor.matmul(
                            out=vps[:],
                            lhsT=xt_sb[k][:, kb * 128 : (kb + 1) * 128],
                            rhs=wv_sb[k][:],
                            start=(k == 0),
                            stop=(k == KC - 1),
                        )
                    nc.vector.tensor_add(
                        out=v_sb[kb][:, :, 0:D],
                        in0=vps[:].rearrange("p (h d) -> p h d", h=NH),
                        in1=bv_sb[:].rearrange("p (h d) -> p h d", h=NH),
                    )
                    nc.vector.memset(v_sb[kb][:, :, D : D + 1], 1.0)

            def attn(p):
                for qb in range(QB):
                    qsl = slice(qb * 512, (qb + 1) * 512)
                    o_ps = [
                        opp.tile([D + 1, 512], F32, tag=f"o{h}", name=f"o{h}")
                        for h in range(2)
                    ]
                    def emit_pv(kb, pt):
                        for h in range(2):
                            nc.tensor.matmul(
                                out=o_ps[h][:],
                                lhsT=v_sb[kb][:, 2 * p + h, :],
                                rhs=pt[:, h * 512 : (h + 1) * 512],
                                start=(kb == 0),
                                stop=(kb == KB - 1),
                                skip_group_check=True,
                            )

                    # PV is emitted one kb behind its exp so the in-order PE
                    # never sits on PV's o-psum wait before issuing the next
                    # QK pair (which would stall the ACT exp pipeline).
                    prev = None
                    for kb in range(KB):
                        ksl = slice(kb * 128, (kb + 1) * 128)
                        st = stp.tile([128, 1024], F32, tag="st", name="st")
                        # S^T chunks for both heads; K=64 matmuls at row
                        # offsets 0 and 64 run concurrently on the PE.
                        for h in range(2):
                            hsl = slice(h * D, (h + 1) * D)
                            nc.tensor.matmul(
                                out=st[:, h * 512 : (h + 1) * 512],
                                lhsT=kt_sb[p][hsl, ksl],
                                rhs=qt_sb[p][hsl, qsl],
                                start=True,
                                stop=True,
                            )
                        pt = ptp.tile([128, 1024], BF16, tag="pt", name="pt")
                        nc.scalar.activation(
                            out=pt[:], in_=st[:], func=Exp, scale=SCALE
                        )
                        if prev is not None:
                            emit_pv(*prev)
                        prev = (kb, pt)
                    emit_pv(*prev)
                    for h in range(2):
                        ot = otp.tile([D + 1, 512], F32, tag="ot", name="ot")
                        nc.vector.tensor_copy(out=ot[:], in_=o_ps[h][:])
                        nc.sync.dma_start(out=out[2 * p + h, :, qsl], in_=ot[:])

            proj_qk(0)
            proj_v()
            attn(0)
            proj_qk(1)
            attn(1)


def build_nc():
    nc = bacc.Bacc(
        "TRN2",
        target_bir_lowering=False,
        debug=False,
        num_devices=NCORES,
        enable_partition_id=False,
    )
    xt = nc.dram_tensor("xt", [C, N], BF16, kind="ExternalInput").ap()
    wqt = nc.dram_tensor("wqt", [C, HD], BF16, kind="ExternalInput").ap()
    wkt = nc.dram_tensor("wkt", [C, HD], BF16, kind="ExternalInput").ap()
    wvt = nc.dram_tensor("wvt", [C, HD], BF16, kind="ExternalInput").ap()
    bq = nc.dram_tensor("bq", [HD], F32, kind="ExternalInput").ap()
    bk = nc.dram_tensor("bk", [HD], F32, kind="ExternalInput").ap()
    bv = nc.dram_tensor("bv", [HD], F32, kind="ExternalInput").ap()
    out = nc.dram_tensor("out", [NH, D + 1, N], F32, kind="ExternalOutput").ap()

    with tile.TileContext(nc) as tc:
        build_kernel(tc, xt, wqt, wkt, wvt, bq, bk, bv, out)
    nc.compile()
    return nc


def shard_inputs(inputs):
    x = np.asarray(inputs["x"], np.float32)
    in_maps = []
    for core in range(NCORES):
        b, g = core // 4, core % 4
        sl = slice(g * HD, (g + 1) * HD)
        in_maps.append(
            {
                "xt": np.ascontiguousarray(x[b].T).astype(ml_dtypes.bfloat16),
                "wqt": np.ascontiguousarray(np.asarray(inputs["Wq"], np.float32)[sl, :].T).astype(ml_dtypes.bfloat16),
                "wkt": np.ascontiguousarray(np.asarray(inputs["Wk"], np.float32)[sl, :].T).astype(ml_dtypes.bfloat16),
                "wvt": np.ascontiguousarray(np.asarray(inputs["Wv"], np.float32)[sl, :].T).astype(ml_dtypes.bfloat16),
                "bq": np.ascontiguousarray(np.asarray(inputs["bq"], np.float32)[sl]),
                "bk": np.ascontiguousarray(np.asarray(inputs["bk"], np.float32)[sl]),
                "bv": np.ascontiguousarray(np.asarray(inputs["bv"], np.float32)[sl]),
            }
        )
    return in_maps


def assemble(results, B=2):
    out = np.zeros((B, N, C), np.float32)
    for core in range(NCORES):
        b, g = core // 4, core % 4
        o = np.asarray(results[core]["out"], np.float32)  # [NH, D+1, N]
        on = o[:, 0:D, :] / o[:, D : D + 1, :]  # normalize by row-sums
        # [h, d, n] -> [n, h*D + d]
        out[b, :, g * HD : (g + 1) * HD] = (
            on.transpose(2, 0, 1).reshape(N, HD)
        )
    return out


_NC_CACHE = None


def _get_nc():
    global _NC_CACHE
    if _NC_CACHE is None:
        _NC_CACHE = build_nc()
    return _NC_CACHE


def kernel(**inputs):
    nc = _get_nc()
    in_maps = shard_inputs(inputs)
    res = run_bass_kernel_spmd(
        nc,
        in_maps,
        core_ids=list(range(NCORES)),
        trace=bool(int(os.environ.get("KERNEL_TRACE", "0"))),
    )
    return assemble(res.results, B=int(np.asarray(inputs["x"]).shape[0]))
